# revision 26
# baseline (speedup 1.0000x reference)
"""CentroidSeparationLoss on 8 Trainium2 NeuronCores.

Strategy (data-parallel over the batch):
  - Shard the 1M rows across 8 cores (125056 rows/core, tail zero-padded with
    out-of-range targets so padded rows contribute nothing).
  - On each core, one streaming pass over its feature shard computes:
      * per-class sums^T [128,64] via PE:  psum += f_tile.T @ onehot_tile
        (f as stationary weights [128,128] bf16 -> fast weight load;
         onehot moving, N=64)
      * per-(subtile,class) counts via PE: psum[1,512] += ones.T @ onehot_wide
      * total sum-of-squares via ACT Square with per-partition accumulator
  - Host gathers the tiny partials (<70KB/core), computes centers and the
    closed-form intra loss  (SSQ - 2*<sums,centers> + sum_c n_c||c_c||^2)/B
    (algebraically identical to mean ||f_i - center_{t_i}||^2, so the second
    pass over features in the reference is not needed), plus the pairwise
    inter hinge loss on the 64 centers.

Engine split per 2048-row tile (DMA-bound at ~4.5us/tile measured):
  ACT: fp32->bf16 cast (Copy) + Square+accum  (~4us)
  DVE: targets convert + one-hot is_equal      (~2.2us)
  PE : 16 FWL loads + 16 N=64 matmuls + counts (~1.5-2.5us)
  GPSIMD: idle (one-time ones memset only)
"""

import numpy as np
import ml_dtypes

import concourse.bacc as bacc
import concourse.mybir as mybir
import concourse.tile as tile
from concourse.bass_utils import run_bass_kernel_spmd

P = 128          # partitions
C = 64           # classes
D = 128          # feature dim
NJ = 16          # subtiles (of 128 rows) per big tile
N_CORES = 8
B_FULL = 1_000_000
ROWS_PER_CORE = 125056           # 977 subtiles of 128 rows
N_BIG = 61                       # 61*16 = 976 subtiles
TAIL_NJ = 1                      # + 1 subtile = 977
MARGIN = 2.0
PAD_CLASS = C                    # out-of-range target for padded rows
CNT_CHUNK = 8                    # subtiles per counts-matmul (N=512 psum bank)
CAST_GP = 5                      # trailing subtiles whose cast runs on GPSIMD
SQ_ACT = 9                       # leading subtiles whose square runs on ACT
SQ_DVE = SQ_ACT  # back-compat alias for tests (split point)

F32 = mybir.dt.float32
BF16 = mybir.dt.bfloat16
I32 = mybir.dt.int32

# bisection flags for HW load debugging
ENABLE_COUNTS = True
ENABLE_COUNTS_OUT = True
ENABLE_SUMS = True
ENABLE_SSQ = True


def kernel_body(tc, outs, ins, tiles_nj):
    """Emit the per-core program.

    ins  = (features [rows, D] f32, targets [rows] i32, iota [P, NJ*C] bf16)
    outs = (out_sums [P, C] f32  (= per-class sums, transposed),
            out_counts [nj_max*C] f32 (per-(subtile-slot, class) counts),
            out_ssq [P, n_tiles] f32)
    tiles_nj = list of subtile counts per tile (sum * P == rows)
    """
    nc = tc.nc
    feat, tgt, iota_in = ins
    out_sums, out_counts, out_ssq = outs
    n_tiles = len(tiles_nj)
    nj_max = max(tiles_nj)
    n_mm = sum(tiles_nj)
    n_cnt = nj_max * C
    n_chunks = (nj_max + CNT_CHUNK - 1) // CNT_CHUNK
    # per counts-chunk total matmul count (for start/stop flags)
    chunk_total = [0] * n_chunks
    for nj in tiles_nj:
        for ci in range((nj + CNT_CHUNK - 1) // CNT_CHUNK):
            chunk_total[ci] += 1

    with (
        tc.tile_pool(name="pf32", bufs=6) as pf32,
        tc.tile_pool(name="pfaug", bufs=4) as pfaug,
        tc.tile_pool(name="poh", bufs=4) as poh,
        tc.tile_pool(name="ptgt", bufs=6) as ptgt,
        tc.tile_pool(name="psq", bufs=2) as psq,
        tc.tile_pool(name="pconst", bufs=1) as pconst,
        tc.tile_pool(name="pout", bufs=1) as pout,
        tc.tile_pool(name="ppsum", bufs=1, space="PSUM") as ppsum,
    ):
        iota_sb = pconst.tile([P, nj_max, C], I32)
        nc.sync.dma_start(
            iota_sb[:, :, :],
            iota_in[:, 0 : nj_max * C].rearrange("p (j c) -> p j c", c=C),
        )
        # 32-wide ones weights: M=32 is the PE tile granularity (M=1 output
        # failed to load on HW); rows of the counts psum are identical, host
        # reads row 0.
        CM = 32
        ones_sb = pconst.tile([P, CM], BF16)
        nc.gpsimd.memset(ones_sb[:, :], 1.0)
        ssq_cols = pconst.tile([P, 2 * n_tiles], F32)
        nc.gpsimd.memset(ssq_cols[:, :], 0.0)
        psum_sumsT = ppsum.tile([P, C], F32)
        psum_cnt = [
            ppsum.tile([CM, min(CNT_CHUNK * C, n_cnt - ci * CNT_CHUNK * C)], F32,
                       name=f"psum_cnt{ci}", tag=f"cnt{ci}")
            for ci in range(n_chunks)
        ]

        mm_idx = 0
        chunk_seen = [0] * n_chunks
        row0 = 0
        for t in range(n_tiles):
            nj = tiles_nj[t]
            rows = P * nj
            fap = feat[row0 : row0 + rows, :].rearrange(
                "(p j) d -> p j d", p=P, j=nj
            )
            tap = tgt[row0 : row0 + rows].rearrange("(p j) -> p j", p=P, j=nj)
            row0 += rows

            f32t = pf32.tile([P, nj, D], F32, tag="f32t")
            nc.sync.dma_start(f32t[:, :, :], fap)
            tgti = ptgt.tile([P, nj], I32, tag="tgti")
            nc.scalar.dma_start(tgti[:, :], tap)

            oh = poh.tile([P, nj, C], BF16, tag="oh")
            nc.vector.tensor_tensor(
                oh[:, :, :],
                iota_sb[:, 0:nj, :],
                tgti[:, :].broadcast_to([P, nj, C]),
                op=mybir.AluOpType.is_equal,
            )

            faug = pfaug.tile([P, nj, D], BF16, tag="faug")
            c_sp = min(nj, nj - CAST_GP) if nj == NJ else nj
            nc.scalar.activation(
                faug[:, 0:c_sp, :],
                f32t[:, 0:c_sp, :],
                mybir.ActivationFunctionType.Copy,
            )
            if nj > c_sp:
                nc.gpsimd.tensor_copy(faug[:, c_sp:nj, :], f32t[:, c_sp:nj, :])
            sq = psq.tile([P, nj, D], BF16, tag="sq")
            if ENABLE_SSQ:
                s_sp = min(nj, SQ_ACT)
                nc.scalar.activation(
                    sq[:, 0:s_sp, :],
                    faug[:, 0:s_sp, :],
                    mybir.ActivationFunctionType.Square,
                    accum_out=ssq_cols[:, 2 * t : 2 * t + 1],
                )
                if nj > s_sp:
                    nc.vector.scalar_tensor_tensor(
                        out=sq[:, s_sp:nj, :],
                        in0=faug[:, s_sp:nj, :],
                        scalar=1.0,
                        in1=faug[:, s_sp:nj, :],
                        op0=mybir.AluOpType.mult,
                        op1=mybir.AluOpType.mult,
                        accum_out=ssq_cols[:, 2 * t + 1 : 2 * t + 2],
                    )

            if ENABLE_SUMS:
                for j in range(nj):
                    nc.tensor.matmul(
                        psum_sumsT[:, :],
                        lhsT=faug[:, j, :],
                        rhs=oh[:, j, :],
                        start=(mm_idx == 0),
                        stop=(mm_idx == n_mm - 1),
                    )
                    mm_idx += 1
            if ENABLE_COUNTS:
                for ci in range((nj + CNT_CHUNK - 1) // CNT_CHUNK):
                    a = ci * CNT_CHUNK
                    b = min(nj, a + CNT_CHUNK)
                    chunk_seen[ci] += 1
                    nc.tensor.matmul(
                        psum_cnt[ci][:, 0 : (b - a) * C],
                        lhsT=ones_sb[:, :],
                        rhs=oh[:, a:b, :],
                        start=(chunk_seen[ci] == 1),
                        stop=(chunk_seen[ci] == chunk_total[ci]),
                    )

        if ENABLE_SUMS:
            sums_sb = pout.tile([P, C], F32)
            nc.vector.tensor_copy(sums_sb[:, :], psum_sumsT[:, :])
            nc.sync.dma_start(out_sums[:, :], sums_sb[:, :])
        if ENABLE_COUNTS and ENABLE_COUNTS_OUT:
            cnt_sb = pout.tile([1, n_cnt], F32)
            for ci in range(n_chunks):
                w = psum_cnt[ci].shape[1]
                nc.vector.tensor_copy(
                    cnt_sb[:, ci * CNT_CHUNK * C : ci * CNT_CHUNK * C + w],
                    psum_cnt[ci][0:1, :],
                )
            nc.sync.dma_start(out_counts[0:1, :], cnt_sb[0:1, :])
        if ENABLE_SSQ:
            nc.sync.dma_start(out_ssq[:, :], ssq_cols[:, :])


def build_program(rows, tiles_nj):
    # Bacc (not raw Bass): its compile() runs generate_event_semaphores,
    # which splits multi-semaphore waits into EventSemaphore instructions —
    # TRN2 instructions (notably direct-2D DMAs) carry at most one wait.
    nc = bacc.Bacc()
    n_tiles = len(tiles_nj)
    nj_max = max(tiles_nj)
    feat = nc.dram_tensor("features", [rows, D], F32, kind="ExternalInput")
    tgt = nc.dram_tensor("targets", [rows], I32, kind="ExternalInput")
    iota_in = nc.dram_tensor("iota", [P, nj_max * C], I32, kind="ExternalInput")
    out_sums = nc.dram_tensor("out_sums", [P, C], F32, kind="ExternalOutput")
    out_counts = nc.dram_tensor("out_counts", [1, nj_max * C], F32,
                                kind="ExternalOutput")
    out_ssq = nc.dram_tensor("out_ssq", [P, 2 * n_tiles], F32, kind="ExternalOutput")
    with tile.TileContext(nc) as tc:
        kernel_body(
            tc,
            (out_sums[:, :], out_counts[:, :], out_ssq[:, :]),
            (feat[:, :], tgt[:], iota_in[:, :]),
            tiles_nj,
        )
    nc.compile()
    return nc


def iota_input(nj_max=NJ):
    row = np.tile(np.arange(C, dtype=np.int32), nj_max)
    return np.broadcast_to(row, (P, nj_max * C)).copy()


_PROGRAM = None


def _get_program():
    global _PROGRAM
    if _PROGRAM is None:
        _PROGRAM = build_program(ROWS_PER_CORE, [NJ] * N_BIG + [TAIL_NJ])
    return _PROGRAM


def make_in_maps(features, targets):
    features = np.asarray(features)
    targets = np.asarray(targets)
    if features.dtype != np.float32:
        features = features.astype(np.float32)
    if targets.dtype != np.int32:
        targets = targets.astype(np.int32)
    iota_np = iota_input()
    in_maps = []
    b = features.shape[0]
    for i in range(N_CORES):
        lo = i * ROWS_PER_CORE
        hi = min((i + 1) * ROWS_PER_CORE, b)
        f = features[lo:hi]
        t = targets[lo:hi]
        pad = ROWS_PER_CORE - (hi - lo)
        if pad:
            f = np.concatenate([f, np.zeros((pad, D), np.float32)])
            t = np.concatenate([t, np.full((pad,), PAD_CLASS, np.int32)])
        in_maps.append({"features": f, "targets": t, "iota": iota_np})
    return in_maps


def reduce_partials(sums_parts, counts_parts, ssq_parts, b):
    """sums_parts: [P,C] (transposed sums); counts_parts: [nj*C];
    ssq_parts: [P, n_tiles]."""
    sums = np.zeros((C, D), np.float64)
    for s in sums_parts:
        sums += s.astype(np.float64).T
    counts = np.zeros(C, np.float64)
    for cp in counts_parts:
        counts += cp.astype(np.float64).reshape(-1, C).sum(axis=0)
    ssq = float(sum(s.astype(np.float64).sum() for s in ssq_parts))

    counts_c = np.maximum(counts, 1.0)
    centers = sums / counts_c[:, None]
    intra = (
        ssq
        - 2.0 * float((sums * centers).sum())
        + float((counts * (centers**2).sum(axis=1)).sum())
    ) / b

    gram = centers @ centers.T
    n2 = np.diag(gram)
    d2 = n2[:, None] + n2[None, :] - 2.0 * gram
    hinge = np.maximum(MARGIN - d2, 0.0)
    w = np.ones((C, C))
    w[1, 2] = 2.0
    upper = np.triu(np.ones((C, C)), k=1)
    n_pairs = C * (C - 1) // 2
    inter = float((w * hinge * upper).sum()) / n_pairs
    return np.float32(intra + inter)


def run(features, targets, trace=False, trace_cores=None):
    nc = _get_program()
    in_maps = make_in_maps(features, targets)
    res = run_bass_kernel_spmd(
        nc,
        in_maps,
        core_ids=list(range(N_CORES)),
        trace=trace,
        trace_cores=trace_cores,
    )
    out = reduce_partials(
        [r["out_sums"] for r in res.results],
        [r["out_counts"] for r in res.results],
        [r["out_ssq"] for r in res.results],
        np.asarray(features).shape[0],
    )
    return out, res


def kernel(features, targets):
    out, _ = run(features, targets)
    return np.array(out, dtype=np.float32)


# revision 27
# speedup vs baseline: 1.1513x; 1.1513x over previous
"""CentroidSeparationLoss on 8 Trainium2 NeuronCores.

Strategy (data-parallel over the batch):
  - Shard the 1M rows across 8 cores (125056 rows/core, tail zero-padded with
    out-of-range targets so padded rows contribute nothing).
  - On each core, one streaming pass over its feature shard computes:
      * per-class sums^T [128,64] via PE:  psum += f_tile.T @ onehot_tile
        (f as stationary weights [128,128] bf16 -> fast weight load;
         onehot moving, N=64)
      * per-(subtile,class) counts via PE: psum[1,512] += ones.T @ onehot_wide
      * total sum-of-squares via ACT Square with per-partition accumulator
  - Host gathers the tiny partials (<70KB/core), computes centers and the
    closed-form intra loss  (SSQ - 2*<sums,centers> + sum_c n_c||c_c||^2)/B
    (algebraically identical to mean ||f_i - center_{t_i}||^2, so the second
    pass over features in the reference is not needed), plus the pairwise
    inter hinge loss on the 64 centers.

Engine split per 2048-row tile (DMA-bound at ~4.5us/tile measured):
  ACT: fp32->bf16 cast (Copy) + Square+accum  (~4us)
  DVE: targets convert + one-hot is_equal      (~2.2us)
  PE : 16 FWL loads + 16 N=64 matmuls + counts (~1.5-2.5us)
  GPSIMD: idle (one-time ones memset only)
"""

import numpy as np
import ml_dtypes

import concourse.bacc as bacc
import concourse.mybir as mybir
import concourse.tile as tile
from concourse.bass_utils import run_bass_kernel_spmd

P = 128          # partitions
C = 64           # classes
D = 128          # feature dim
NJ = 16          # subtiles (of 128 rows) per big tile
N_CORES = 8
B_FULL = 1_000_000
ROWS_PER_CORE = 125056           # 977 subtiles of 128 rows
N_BIG = 61                       # 61*16 = 976 subtiles
TAIL_NJ = 1                      # + 1 subtile = 977
MARGIN = 2.0
PAD_CLASS = C                    # out-of-range target for padded rows
CNT_CHUNK = 8                    # subtiles per counts-matmul (N=512 psum bank)
CAST_GP = 3                      # trailing subtiles whose cast runs on GPSIMD
SQ_ACT = 8                       # leading subtiles whose square runs on ACT
SQ_DVE = SQ_ACT  # back-compat alias for tests (split point)

F32 = mybir.dt.float32
BF16 = mybir.dt.bfloat16
I32 = mybir.dt.int32

# bisection flags for HW load debugging
ENABLE_COUNTS = True
ENABLE_COUNTS_OUT = True
ENABLE_SUMS = True
ENABLE_SSQ = True


def kernel_body(tc, outs, ins, tiles_nj):
    """Emit the per-core program.

    ins  = (features [rows, D] f32, targets [rows] i32, iota [P, NJ*C] bf16)
    outs = (out_sums [P, C] f32  (= per-class sums, transposed),
            out_counts [nj_max*C] f32 (per-(subtile-slot, class) counts),
            out_ssq [P, n_tiles] f32)
    tiles_nj = list of subtile counts per tile (sum * P == rows)
    """
    nc = tc.nc
    feat, tgt, iota_in = ins
    out_sums, out_counts, out_ssq = outs
    n_tiles = len(tiles_nj)
    nj_max = max(tiles_nj)
    n_mm = sum(tiles_nj)
    n_cnt = nj_max * C
    n_chunks = (nj_max + CNT_CHUNK - 1) // CNT_CHUNK
    # per counts-chunk total matmul count (for start/stop flags)
    chunk_total = [0] * n_chunks
    for nj in tiles_nj:
        for ci in range((nj + CNT_CHUNK - 1) // CNT_CHUNK):
            chunk_total[ci] += 1

    with (
        tc.tile_pool(name="pf32", bufs=6) as pf32,
        tc.tile_pool(name="pfaug", bufs=4) as pfaug,
        tc.tile_pool(name="poh", bufs=4) as poh,
        tc.tile_pool(name="ptgt", bufs=6) as ptgt,
        tc.tile_pool(name="psq", bufs=2) as psq,
        tc.tile_pool(name="pconst", bufs=1) as pconst,
        tc.tile_pool(name="pout", bufs=1) as pout,
        tc.tile_pool(name="ppsum", bufs=1, space="PSUM") as ppsum,
    ):
        iota_sb = pconst.tile([P, nj_max, C], I32)
        nc.sync.dma_start(
            iota_sb[:, :, :],
            iota_in[:, 0 : nj_max * C].rearrange("p (j c) -> p j c", c=C),
        )
        # 32-wide ones weights: M=32 is the PE tile granularity (M=1 output
        # failed to load on HW); rows of the counts psum are identical, host
        # reads row 0.
        CM = 32
        ones_sb = pconst.tile([P, CM], BF16)
        nc.gpsimd.memset(ones_sb[:, :], 1.0)
        ssq_cols = pconst.tile([P, 2 * n_tiles], F32)
        nc.gpsimd.memset(ssq_cols[:, :], 0.0)
        psum_sumsT = ppsum.tile([P, C], F32)
        psum_cnt = [
            ppsum.tile([CM, min(CNT_CHUNK * C, n_cnt - ci * CNT_CHUNK * C)], F32,
                       name=f"psum_cnt{ci}", tag=f"cnt{ci}")
            for ci in range(n_chunks)
        ]

        mm_idx = 0
        chunk_seen = [0] * n_chunks
        row0 = 0
        for t in range(n_tiles):
            nj = tiles_nj[t]
            rows = P * nj
            fap = feat[row0 : row0 + rows, :].rearrange(
                "(p j) d -> p j d", p=P, j=nj
            )
            tap = tgt[row0 : row0 + rows].rearrange("(p j) -> p j", p=P, j=nj)
            row0 += rows

            f32t = pf32.tile([P, nj, D], F32, tag="f32t")
            nc.sync.dma_start(f32t[:, :, :], fap)
            tgti = ptgt.tile([P, nj], I32, tag="tgti")
            nc.scalar.dma_start(tgti[:, :], tap)

            oh = poh.tile([P, nj, C], BF16, tag="oh")
            nc.vector.tensor_tensor(
                oh[:, :, :],
                iota_sb[:, 0:nj, :],
                tgti[:, :].broadcast_to([P, nj, C]),
                op=mybir.AluOpType.is_equal,
            )

            faug = pfaug.tile([P, nj, D], BF16, tag="faug")
            c_sp = min(nj, nj - CAST_GP) if nj == NJ else nj
            nc.scalar.activation(
                faug[:, 0:c_sp, :],
                f32t[:, 0:c_sp, :],
                mybir.ActivationFunctionType.Copy,
            )
            if nj > c_sp:
                nc.gpsimd.tensor_copy(faug[:, c_sp:nj, :], f32t[:, c_sp:nj, :])
            sq = psq.tile([P, nj, D], BF16, tag="sq")
            if ENABLE_SSQ:
                s_sp = min(nj, SQ_ACT)
                nc.scalar.activation(
                    sq[:, 0:s_sp, :],
                    faug[:, 0:s_sp, :],
                    mybir.ActivationFunctionType.Square,
                    accum_out=ssq_cols[:, 2 * t : 2 * t + 1],
                )
                if nj > s_sp:
                    nc.vector.scalar_tensor_tensor(
                        out=sq[:, s_sp:nj, :],
                        in0=faug[:, s_sp:nj, :],
                        scalar=1.0,
                        in1=faug[:, s_sp:nj, :],
                        op0=mybir.AluOpType.mult,
                        op1=mybir.AluOpType.mult,
                        accum_out=ssq_cols[:, 2 * t + 1 : 2 * t + 2],
                    )

            if ENABLE_SUMS:
                for j in range(nj):
                    nc.tensor.matmul(
                        psum_sumsT[:, :],
                        lhsT=faug[:, j, :],
                        rhs=oh[:, j, :],
                        start=(mm_idx == 0),
                        stop=(mm_idx == n_mm - 1),
                    )
                    mm_idx += 1
            if ENABLE_COUNTS:
                for ci in range((nj + CNT_CHUNK - 1) // CNT_CHUNK):
                    a = ci * CNT_CHUNK
                    b = min(nj, a + CNT_CHUNK)
                    chunk_seen[ci] += 1
                    nc.tensor.matmul(
                        psum_cnt[ci][:, 0 : (b - a) * C],
                        lhsT=ones_sb[:, :],
                        rhs=oh[:, a:b, :],
                        start=(chunk_seen[ci] == 1),
                        stop=(chunk_seen[ci] == chunk_total[ci]),
                    )

        if ENABLE_SUMS:
            sums_sb = pout.tile([P, C], F32)
            nc.vector.tensor_copy(sums_sb[:, :], psum_sumsT[:, :])
            nc.sync.dma_start(out_sums[:, :], sums_sb[:, :])
        if ENABLE_COUNTS and ENABLE_COUNTS_OUT:
            cnt_sb = pout.tile([1, n_cnt], F32)
            for ci in range(n_chunks):
                w = psum_cnt[ci].shape[1]
                nc.vector.tensor_copy(
                    cnt_sb[:, ci * CNT_CHUNK * C : ci * CNT_CHUNK * C + w],
                    psum_cnt[ci][0:1, :],
                )
            nc.sync.dma_start(out_counts[0:1, :], cnt_sb[0:1, :])
        if ENABLE_SSQ:
            nc.sync.dma_start(out_ssq[:, :], ssq_cols[:, :])


def build_program(rows, tiles_nj):
    # Bacc (not raw Bass): its compile() runs generate_event_semaphores,
    # which splits multi-semaphore waits into EventSemaphore instructions —
    # TRN2 instructions (notably direct-2D DMAs) carry at most one wait.
    nc = bacc.Bacc()
    n_tiles = len(tiles_nj)
    nj_max = max(tiles_nj)
    feat = nc.dram_tensor("features", [rows, D], F32, kind="ExternalInput")
    tgt = nc.dram_tensor("targets", [rows], I32, kind="ExternalInput")
    iota_in = nc.dram_tensor("iota", [P, nj_max * C], I32, kind="ExternalInput")
    out_sums = nc.dram_tensor("out_sums", [P, C], F32, kind="ExternalOutput")
    out_counts = nc.dram_tensor("out_counts", [1, nj_max * C], F32,
                                kind="ExternalOutput")
    out_ssq = nc.dram_tensor("out_ssq", [P, 2 * n_tiles], F32, kind="ExternalOutput")
    with tile.TileContext(nc) as tc:
        kernel_body(
            tc,
            (out_sums[:, :], out_counts[:, :], out_ssq[:, :]),
            (feat[:, :], tgt[:], iota_in[:, :]),
            tiles_nj,
        )
    nc.compile()
    return nc


def iota_input(nj_max=NJ):
    row = np.tile(np.arange(C, dtype=np.int32), nj_max)
    return np.broadcast_to(row, (P, nj_max * C)).copy()


_PROGRAM = None


def _get_program():
    global _PROGRAM
    if _PROGRAM is None:
        _PROGRAM = build_program(ROWS_PER_CORE, [NJ] * N_BIG + [TAIL_NJ])
    return _PROGRAM


def make_in_maps(features, targets):
    features = np.asarray(features)
    targets = np.asarray(targets)
    if features.dtype != np.float32:
        features = features.astype(np.float32)
    if targets.dtype != np.int32:
        targets = targets.astype(np.int32)
    iota_np = iota_input()
    in_maps = []
    b = features.shape[0]
    for i in range(N_CORES):
        lo = i * ROWS_PER_CORE
        hi = min((i + 1) * ROWS_PER_CORE, b)
        f = features[lo:hi]
        t = targets[lo:hi]
        pad = ROWS_PER_CORE - (hi - lo)
        if pad:
            f = np.concatenate([f, np.zeros((pad, D), np.float32)])
            t = np.concatenate([t, np.full((pad,), PAD_CLASS, np.int32)])
        in_maps.append({"features": f, "targets": t, "iota": iota_np})
    return in_maps


def reduce_partials(sums_parts, counts_parts, ssq_parts, b):
    """sums_parts: [P,C] (transposed sums); counts_parts: [nj*C];
    ssq_parts: [P, n_tiles]."""
    sums = np.zeros((C, D), np.float64)
    for s in sums_parts:
        sums += s.astype(np.float64).T
    counts = np.zeros(C, np.float64)
    for cp in counts_parts:
        counts += cp.astype(np.float64).reshape(-1, C).sum(axis=0)
    ssq = float(sum(s.astype(np.float64).sum() for s in ssq_parts))

    counts_c = np.maximum(counts, 1.0)
    centers = sums / counts_c[:, None]
    intra = (
        ssq
        - 2.0 * float((sums * centers).sum())
        + float((counts * (centers**2).sum(axis=1)).sum())
    ) / b

    gram = centers @ centers.T
    n2 = np.diag(gram)
    d2 = n2[:, None] + n2[None, :] - 2.0 * gram
    hinge = np.maximum(MARGIN - d2, 0.0)
    w = np.ones((C, C))
    w[1, 2] = 2.0
    upper = np.triu(np.ones((C, C)), k=1)
    n_pairs = C * (C - 1) // 2
    inter = float((w * hinge * upper).sum()) / n_pairs
    return np.float32(intra + inter)


def run(features, targets, trace=False, trace_cores=None):
    nc = _get_program()
    in_maps = make_in_maps(features, targets)
    res = run_bass_kernel_spmd(
        nc,
        in_maps,
        core_ids=list(range(N_CORES)),
        trace=trace,
        trace_cores=trace_cores,
    )
    out = reduce_partials(
        [r["out_sums"] for r in res.results],
        [r["out_counts"] for r in res.results],
        [r["out_ssq"] for r in res.results],
        np.asarray(features).shape[0],
    )
    return out, res


def kernel(features, targets):
    out, _ = run(features, targets)
    return np.array(out, dtype=np.float32)
